# revision 27
# baseline (speedup 1.0000x reference)
"""Trainium2 Bass kernel for nn_GCN1 (2-layer GCN + MLP head), v2.

Contract: kernel(**inputs) takes FULL unsharded numpy inputs (as produced by
setup_inputs) and returns the FULL [64, 10] output.  Edges are partitioned by
destination node across 8 NeuronCores (segment-sum per shard needs no
all-reduce); node features are exchanged between the two graph-conv layers
with one AllGather and the MLP-head partial sums with one AllReduce.

v2 structure (driven by the v1 trace: SWDGE descriptor generation on GpSimd
was 315 us busy and fully serial with everything else):
  * prop1 needs no gather at all: its message stream ms1[t] = xs0[src[t]] is
    a pure layout of the (host-prescaled) input features, so the host
    materializes it per-core and the device just streams it with HWDGE
    dma_start + DVE round adds.
  * prop2's gather descriptors are generated AHEAD of time with
    dma_gather(prepare_only=True) while prop1 streams and the AllGather runs;
    trigger_dma fires them the moment y1full lands.  The SWDGE descriptor
    ring (dynamic_dma_scratch_size) is enlarged to hold the whole prop's
    descriptors so generation never stalls on drain.
  * GCN math collapse (b0 == 0):  conv0+leaky+W1 folds to
    y1 = f1*(alpha*s + beta*|s|) = fab * Lrelu_a(s), a=(alpha-beta)/(alpha+beta),
    computed as ONE scalar-engine Lrelu (alpha from an input-derived SBUF
    scalar) + ONE DVE multiply by a host-precomputed per-node broadcast table.
  * h1 = leaky(deg_in^-0.5*agg1 + b1) is one DVE multiply + one ACT Lrelu.
  * MLP head runs in bf16 (LDWEIGHTS halves); each layer's PSUM->SBUF copy,
    bias add and leaky fuse into a single ACT Lrelu.

Round layout (unchanged from v1): round j holds the j-th in-edge of every
destination node, node-major (nodes degree-sorted descending within each
shard), padded to 128 tokens; the segment-sum is one full-width DVE add per
round-segment onto a resident SBUF accumulator.  Round 0 covers every node,
so it is a tensor_copy (no memset needed).
"""

import numpy as np
import ml_dtypes

N = 15828          # real node count (hardcoded per problem spec)
NP = 16384         # padded node count = 8 * 2048
S = 2048           # nodes per core shard
SJ = S // 128      # 16 shard blocks of 128 nodes
B = 64             # batch (propagation payload channels); 64 f32 = 256 bytes
NCORES = 8
HID = 100
TILE1 = 2048       # prop1 stream tile (tokens)
TILE2 = 8192       # prop2 gather tile (tokens, max per dma_gather call)
ZROW = S - 1       # guaranteed-pad (all-zero) table row in shard 0

NEG = 0.01
LA = (1.0 + NEG) / 2.0   # 0.505
LB = (1.0 - NEG) / 2.0   # 0.495


# ----------------------------------------------------------------------------
# Host-side graph preprocessing: pure index/layout work + per-node constants.
# ----------------------------------------------------------------------------

def _balance_nodes(deg_in):
    """Assign nodes to 8 bins of <=2048 balancing total in-edges, then order
    each bin by in-degree descending.  Returns new_label[old] in [0, NP)."""
    order = np.argsort(-deg_in, kind="stable")
    new_label = np.empty(N, dtype=np.int64)
    pos = np.zeros(NCORES, dtype=np.int64)
    edges = np.zeros(NCORES, dtype=np.int64)
    for start in range(0, N, NCORES):
        blk = order[start:start + NCORES]
        bins = np.argsort(edges, kind="stable")
        for i, n in enumerate(blk):
            k = int(bins[i])
            new_label[n] = k * S + pos[k]
            pos[k] += 1
            edges[k] += deg_in[n]
    assert pos.max() <= ZROW, pos  # keep ZROW free as the zero pad row
    return new_label


def _idx_layout(v, cols):
    """Pack int token-index vector v (len = 16*cols) into the SWDGE idx
    layout: [16, cols] with token i at [i % 16, i // 16], replicated to
    128 partitions."""
    a = np.asarray(v, dtype=np.int16).reshape(cols, 16).T  # [16, cols]
    return np.tile(a, (NCORES, 1)).copy()  # [128, cols]


def _bcast_nodes(vals):
    """Per-shard-node vector [S] -> [128, SJ*B] f32 broadcast table matching
    the agg tile layout (node = 128*j + p at [p, j, :])."""
    a = vals.reshape(SJ, 128).T.astype(np.float32)           # [128, SJ]
    return np.repeat(a[:, :, None], B, axis=2).reshape(128, SJ * B).copy()


def _prep(in_feat, edge_index, W0, b0, W1, b1, lw0, lb0, lw2, lb2, lw3, lb3):
    assert not np.asarray(b0).any(), "kernel assumes b0 == 0 (GCN collapse)"
    src = np.asarray(edge_index[0], dtype=np.int64)
    dst = np.asarray(edge_index[1], dtype=np.int64)

    deg_out = np.maximum(np.bincount(src, minlength=N), 1)
    deg_in = np.maximum(np.bincount(dst, minlength=N), 1)

    new_label = _balance_nodes(deg_in.copy())
    src_n = new_label[src]
    dst_n = new_label[dst]

    # padded per-node arrays in new labels
    xs0 = np.zeros((NP, B), dtype=np.float32)
    xs0[new_label] = (np.asarray(in_feat, dtype=np.float32)[:, :, 0]
                      * (deg_out.astype(np.float64) ** -0.5)[:, None])
    dego = np.ones(NP, dtype=np.float64)
    dego[new_label] = deg_out
    degi = np.ones(NP, dtype=np.float64)
    degi[new_label] = deg_in
    lw0n = np.zeros((HID, NP), dtype=np.float32)
    lw0n[:, new_label] = np.asarray(lw0, dtype=np.float32)

    # ---- round-layout token streams ----
    csr = []       # per core: (indptr[S+1], srcs sorted by dst)
    for k in range(NCORES):
        m = (dst_n // S) == k
        dk = dst_n[m] - k * S
        sk = src_n[m]
        o = np.argsort(dk, kind="stable")
        dk, sk = dk[o], sk[o]
        indptr = np.zeros(S + 1, dtype=np.int64)
        np.add.at(indptr, dk + 1, 1)
        indptr = np.cumsum(indptr)
        csr.append((indptr, sk))

    degs_local = [np.diff(c[0]) for c in csr]
    maxdeg = int(max(d.max() for d in degs_local))
    Mhat = [max(int((d > j).sum()) for d in degs_local) for j in range(maxdeg)]
    C = [-(-m // 128) for m in Mhat]      # round width in 128-token blocks
    rb = np.concatenate([[0], np.cumsum(np.array(C) * 128)])  # round bases
    e_pad = int(rb[-1])

    def mk_tiles(tile):
        tiles = []
        off = 0
        while off < e_pad:
            tiles.append(int(min(tile, e_pad - off)))
            off += tile
        segs = []  # per tile: list of (msg_col_a, msg_col_b, agg_col, round_j)
        tcol = 0
        for tlen in tiles:
            t_lo, t_hi = tcol, tcol + tlen // 128
            out = []
            for j in range(maxdeg):
                r_lo, r_hi = int(rb[j]) // 128, int(rb[j + 1]) // 128
                a, b_ = max(r_lo, t_lo), min(r_hi, t_hi)
                if a < b_:
                    out.append((int(a - t_lo), int(b_ - t_lo),
                                int(a - r_lo), int(j)))
            segs.append(tuple(out))
            tcol = t_hi
        return tuple(tiles), tuple(segs)

    tiles1, segs1 = mk_tiles(TILE1)
    tiles2, segs2 = mk_tiles(TILE2)
    b1_zero = not np.asarray(b1).any()

    # GCN collapse scalars (host; value-dependent things ship as inputs)
    w0v = np.asarray(W0, dtype=np.float64).reshape(-1)
    w1v = np.asarray(W1, dtype=np.float64).reshape(-1)
    alpha = LA * float((w0v * w1v).sum())
    beta = LB * float((np.abs(w0v) * w1v).sum())
    apb = alpha + beta
    assert abs(apb) > 1e-12, "degenerate W0/W1 (alpha+beta == 0)"
    slope = (alpha - beta) / apb

    lw2T = np.zeros((128, HID), dtype=np.float32)
    lw2T[:HID] = np.asarray(lw2, dtype=np.float32).T
    lw3T = np.zeros((128, 16), dtype=np.float32)
    lw3T[:HID, :10] = np.asarray(lw3, dtype=np.float32).T

    lbias = np.zeros((128, 4), dtype=np.float32)
    lbias[:HID, 0] = np.asarray(lb0, dtype=np.float32)
    lbias[:HID, 1] = np.asarray(lb2, dtype=np.float32)
    lbias[:10, 2] = np.asarray(lb3, dtype=np.float32)

    svec = np.zeros((128, 8), dtype=np.float32)
    svec[:, 0] = np.float32(slope)
    svec[:, 1] = np.float32(np.asarray(b1).reshape(-1)[0])

    in_maps = []
    for k in range(NCORES):
        indptr, sk = csr[k]
        d = degs_local[k]
        tok = np.full(e_pad, ZROW, dtype=np.int64)
        for j in range(maxdeg):
            has = np.nonzero(d > j)[0]       # prefix of labels (deg-sorted)
            tok[int(rb[j]):int(rb[j]) + has.size] = sk[indptr[has] + j]

        # prop1 message stream: xs0[tok] in gather output layout
        # (token i -> partition i%128, column i//128)
        ms1 = xs0[tok].reshape(e_pad // 128, 128, B)
        ms1 = np.ascontiguousarray(ms1.transpose(1, 0, 2)).reshape(128, -1)

        gidx = _idx_layout(tok, e_pad // 16)

        g = np.arange(S) + k * S
        f1 = (dego[g] * degi[g]) ** -0.5
        faa = _bcast_nodes(alpha * f1)
        fbb = _bcast_nodes(beta * f1)
        dib = _bcast_nodes(degi[g] ** -0.5)

        blk = lw0n[:, k * S:(k + 1) * S].T          # [2048, 100]
        blk = blk.reshape(SJ, 128, HID).transpose(1, 0, 2).reshape(128, SJ * HID)

        in_maps.append({
            "gidx": gidx,
            "ms1": ms1,
            "faa": faa,
            "fbb": fbb,
            "dib": dib,
            "svec": svec,
            "lbias": lbias,
            "lw0T": np.ascontiguousarray(blk).astype(ml_dtypes.bfloat16),
            "lw2T": lw2T.astype(ml_dtypes.bfloat16),
            "lw3T": lw3T.astype(ml_dtypes.bfloat16),
        })
    return in_maps, (e_pad, tiles1, segs1, tiles2, segs2, b1_zero)


# ----------------------------------------------------------------------------
# Bass program
# ----------------------------------------------------------------------------

def _build(plan):
    import concourse.bacc as bacc
    import concourse.mybir as mybir
    import concourse.tile as tile

    e_pad, tiles1, segs1, tiles2, segs2, b1_zero = plan
    f32 = mybir.dt.float32
    f16 = mybir.dt.float16
    bf16 = mybir.dt.bfloat16
    i16 = mybir.dt.int16
    AL = mybir.AluOpType
    ACT = mybir.ActivationFunctionType
    icols = e_pad // 16

    # descriptor ring sized to hold all prop2 gather descriptors (2 per token)
    scratch = 73728
    nc = bacc.Bacc("TRN2", target_bir_lowering=False, debug=False,
                   num_devices=NCORES, num_swdge_queues=4,
                   dynamic_dma_scratch_size=scratch)

    gidx_d = nc.dram_tensor("gidx", [128, icols], i16, kind="ExternalInput")
    ms1_d = nc.dram_tensor("ms1", [128, (e_pad // 128) * B], f32,
                           kind="ExternalInput")
    faa_d = nc.dram_tensor("faa", [128, SJ * B], f32, kind="ExternalInput")
    fbb_d = nc.dram_tensor("fbb", [128, SJ * B], f32, kind="ExternalInput")
    dib_d = nc.dram_tensor("dib", [128, SJ * B], f32, kind="ExternalInput")
    svec_d = nc.dram_tensor("svec", [128, 8], f32, kind="ExternalInput")
    lbias_d = nc.dram_tensor("lbias", [128, 4], f32, kind="ExternalInput")
    lw0T_d = nc.dram_tensor("lw0T", [128, SJ * HID], bf16, kind="ExternalInput")
    lw2T_d = nc.dram_tensor("lw2T", [128, HID], bf16, kind="ExternalInput")
    lw3T_d = nc.dram_tensor("lw3T", [128, 16], bf16, kind="ExternalInput")
    out_d = nc.dram_tensor("out", [10, B], f32, kind="ExternalOutput")

    y1in_d = nc.dram_tensor("y1in", [S, B], f32)
    y1full_d = nc.dram_tensor("y1full", [NP, B], f32, addr_space="Shared")
    hpin_d = nc.dram_tensor("hpin", [HID, B], f32)
    hpout_d = nc.dram_tensor("hpout", [HID, B], f32, addr_space="Shared")

    groups = [list(range(NCORES))]

    with tile.TileContext(nc, trace_sim=False) as tc:
        with (
            tc.tile_pool(name="const", bufs=1) as cpool,
            tc.tile_pool(name="ms1", bufs=2) as fpool,
            tc.tile_pool(name="psum", bufs=1, space="PSUM") as ppool,
        ):
            # gather indices first: prop2 descriptor prep depends only on them
            gix = cpool.tile([128, icols], i16)
            nc.sync.dma_start(gix[:], gidx_d.ap())

            # ---- prop2 descriptor prep (generation only; fires later) ----
            msg2 = [cpool.tile([128, tl // 128, B], f32, name=f"msg2_{t}")
                    for t, tl in enumerate(tiles2)]
            sems = [nc.alloc_semaphore(f"gsem{t}") for t in range(len(tiles2))]

            def prep(t):
                tl = tiles2[t]
                tok0 = sum(tiles2[:t])
                nc.gpsimd.dma_gather(
                    msg2[t][:, :tl // 128, :], y1full_d.ap(),
                    gix[:, tok0 // 16:(tok0 + tl) // 16],
                    tl, tl, B, queue_num=t % 4, single_packet=False,
                    prepare_only=True, sem=sems[t])

            # one big prep per queue (per-queue ring = scratch/4 descriptors);
            # generation parallelizes across SWDGE queues and completes before
            # the AllGather blocks gpsimd.  The small tail tile runs as a
            # normal gather after the triggers.
            n_prep = min(4, len(tiles2))
            for t in range(n_prep):
                prep(t)

            # ---- small constant loads (sync engine, overlap prep) ----
            faa = cpool.tile([128, SJ * B], f32)
            nc.sync.dma_start(faa[:], faa_d.ap())
            fbb = cpool.tile([128, SJ * B], f32)
            nc.sync.dma_start(fbb[:], fbb_d.ap())
            dib = cpool.tile([128, SJ * B], f32)
            nc.sync.dma_start(dib[:], dib_d.ap())
            sv = cpool.tile([128, 8], f32)
            nc.sync.dma_start(sv[:], svec_d.ap())
            lb_sb = cpool.tile([128, 4], f32)
            nc.sync.dma_start(lb_sb[:], lbias_d.ap())
            lw0T_sb = cpool.tile([128, SJ * HID], bf16)
            nc.sync.dma_start(lw0T_sb[:], lw0T_d.ap())
            lw2T_sb = cpool.tile([128, HID], bf16)
            nc.sync.dma_start(lw2T_sb[:], lw2T_d.ap())
            lw3T_sb = cpool.tile([128, 16], bf16)
            nc.sync.dma_start(lw3T_sb[:], lw3T_d.ap())

            def seg_reduce(agg, mt, segs_t):
                for (a, b_, c, j) in segs_t:
                    dstv = agg[:, c:c + (b_ - a), :]
                    if j == 0:
                        nc.vector.tensor_copy(dstv, mt[:, a:b_, :])
                    else:
                        nc.vector.tensor_tensor(dstv, dstv, mt[:, a:b_, :],
                                                AL.add)

            # ---- prop1: host-built message stream, no gather ----
            agg0 = cpool.tile([128, SJ, B], f32)
            tok = 0
            for t, tl in enumerate(tiles1):
                blk = tl // 128
                ft = fpool.tile([128, TILE1 // 128, B], f32, tag="ms1")
                nc.sync.dma_start(
                    ft[:, :blk, :].rearrange("p c b -> p (c b)"),
                    ms1_d.ap()[:, (tok // 128) * B:((tok + tl) // 128) * B])
                seg_reduce(agg0, ft, segs1[t])
                tok += tl

            # ---- y1 = faa*agg0 + fbb*|agg0| ----
            y1 = cpool.tile([128, SJ, B], f32)
            a0f = agg0[:].rearrange("p j b -> p (j b)")
            y1f = y1[:].rearrange("p j b -> p (j b)")
            tmp0 = cpool.tile([128, SJ * B], f32)
            nc.scalar.activation(tmp0[:], a0f, ACT.Abs)
            nc.vector.tensor_tensor(tmp0[:], tmp0[:], fbb[:], AL.mult)
            nc.vector.tensor_tensor(y1f, a0f, faa[:], AL.mult)
            nc.vector.tensor_tensor(y1f, y1f, tmp0[:], AL.add)
            nc.sync.dma_start(y1in_d.ap().rearrange("(j p) m -> p j m", p=128),
                              y1[:])

            nc.gpsimd.collective_compute(
                "AllGather", AL.bypass, replica_groups=groups,
                ins=[y1in_d.ap().opt()], outs=[y1full_d.ap().opt()])

            for q in range(n_prep):
                nc.gpsimd.trigger_dma(count=None, queue_num=q)
            for t in range(n_prep, len(tiles2)):
                tl = tiles2[t]
                tok0 = sum(tiles2[:t])
                nc.gpsimd.dma_gather(
                    msg2[t][:, :tl // 128, :], y1full_d.ap(),
                    gix[:, tok0 // 16:(tok0 + tl) // 16],
                    tl, tl, B, queue_num=t % 4, single_packet=False)

            # ---- prop2 segment sums ----
            agg1 = cpool.tile([128, SJ, B], f32)
            for t in range(len(tiles2)):
                seg_reduce(agg1, msg2[t], segs2[t])

            # ---- h1 = leaky(dib * agg1 + b1), emitted in bf16 ----
            def leaky_inplace(x_ap, tmp_ap):
                # x = LA*x + LB*|x|
                nc.scalar.activation(tmp_ap, x_ap, ACT.Abs)
                nc.vector.tensor_scalar(tmp_ap, tmp_ap, float(LB), None,
                                        AL.mult)
                nc.vector.tensor_scalar(x_ap, x_ap, float(LA), None, AL.mult)
                nc.vector.tensor_tensor(x_ap, x_ap, tmp_ap, AL.add)

            tmp = cpool.tile([128, SJ * B], f32)
            a1f = agg1[:].rearrange("p j b -> p (j b)")
            nc.vector.tensor_tensor(tmp[:], a1f, dib[:], AL.mult)
            if not b1_zero:
                nc.vector.tensor_scalar(tmp[:], tmp[:], sv[:, 1:2], None,
                                        AL.add)
            leaky_inplace(tmp[:], tmp0[:])
            h1b = cpool.tile([128, SJ, B], bf16)
            h1bf = h1b[:].rearrange("p j b -> p (j b)")
            nc.vector.tensor_copy(h1bf, tmp[:])

            # ---- head: partial = sum_n lw0T[n,:]^T outer h1[n,:] ----
            ps = ppool.tile([HID, B], f32)
            for j in range(SJ):
                nc.tensor.matmul(ps[:], lhsT=lw0T_sb[:, j * HID:(j + 1) * HID],
                                 rhs=h1b[:, j, :], start=(j == 0),
                                 stop=(j == SJ - 1))
            hp = cpool.tile([HID, B], f32)
            nc.vector.tensor_copy(hp[:], ps[:])
            nc.sync.dma_start(hpin_d.ap(), hp[:])
            nc.gpsimd.collective_compute(
                "AllReduce", AL.add, replica_groups=groups,
                ins=[hpin_d.ap().opt()], outs=[hpout_d.ap().opt()])

            tz = cpool.tile([HID, B], f32)
            z0_32 = cpool.tile([HID, B], f32)
            nc.sync.dma_start(z0_32[:], hpout_d.ap())
            nc.vector.tensor_scalar(z0_32[:], z0_32[:], lb_sb[:HID, 0:1],
                                    None, AL.add)
            leaky_inplace(z0_32[:], tz[:])
            z0 = cpool.tile([HID, B], bf16)
            nc.vector.tensor_copy(z0[:], z0_32[:])

            ps2 = ppool.tile([HID, B], f32)
            nc.tensor.matmul(ps2[:], lhsT=lw2T_sb[:HID, :], rhs=z0[:],
                             start=True, stop=True)
            z1_32 = cpool.tile([HID, B], f32)
            nc.vector.tensor_copy(z1_32[:], ps2[:])
            nc.vector.tensor_scalar(z1_32[:], z1_32[:], lb_sb[:HID, 1:2],
                                    None, AL.add)
            leaky_inplace(z1_32[:], tz[:])
            z1 = cpool.tile([HID, B], bf16)
            nc.vector.tensor_copy(z1[:], z1_32[:])

            ps3 = ppool.tile([10, B], f32)
            nc.tensor.matmul(ps3[:], lhsT=lw3T_sb[:HID, 0:10], rhs=z1[:],
                             start=True, stop=True)
            z2 = cpool.tile([10, B], f32)
            nc.vector.tensor_copy(z2[:], ps3[:])
            nc.vector.tensor_scalar(z2[:], z2[:], lb_sb[:10, 2:3], None,
                                    AL.add)
            tz2 = cpool.tile([10, B], f32)
            leaky_inplace(z2[:], tz2[:])
            nc.sync.dma_start(out_d.ap(), z2[:])

    nc.compile()
    return nc


_BUILD_CACHE = {}
LAST_RESULTS = None  # BassKernelResults from the most recent run (for test.py)
RUN_KWARGS = {}      # extra kwargs for run_bass_kernel_spmd (test.py may set trace)


def kernel(**inputs) -> np.ndarray:
    global LAST_RESULTS
    from concourse.bass_utils import run_bass_kernel_spmd

    in_maps, plan = _prep(**inputs)
    if plan not in _BUILD_CACHE:
        _BUILD_CACHE[plan] = _build(plan)
    nc = _BUILD_CACHE[plan]

    res = run_bass_kernel_spmd(nc, in_maps, core_ids=list(range(NCORES)),
                               **RUN_KWARGS)
    LAST_RESULTS = res
    out = res.results[0]["out"]  # [10, 64]
    return np.ascontiguousarray(out.T.astype(np.float32))


# revision 34
# speedup vs baseline: 1.0699x; 1.0699x over previous
"""Trainium2 Bass kernel for nn_GCN1 (2-layer GCN + MLP head), v2.

Contract: kernel(**inputs) takes FULL unsharded numpy inputs (as produced by
setup_inputs) and returns the FULL [64, 10] output.  Edges are partitioned by
destination node across 8 NeuronCores (segment-sum per shard needs no
all-reduce); node features are exchanged between the two graph-conv layers
with one AllGather and the MLP-head partial sums with one AllReduce.

v2 structure (driven by the v1 trace: SWDGE descriptor generation on GpSimd
was 315 us busy and fully serial with everything else):
  * prop1 needs no gather at all: its message stream ms1[t] = xs0[src[t]] is
    a pure layout of the (host-prescaled) input features, so the host
    materializes it per-core and the device just streams it with HWDGE
    dma_start + DVE round adds.
  * prop2's gather descriptors are generated AHEAD of time with
    dma_gather(prepare_only=True) while prop1 streams and the AllGather runs;
    trigger_dma fires them the moment y1full lands.  The SWDGE descriptor
    ring (dynamic_dma_scratch_size) is enlarged to hold the whole prop's
    descriptors so generation never stalls on drain.
  * GCN math collapse (b0 == 0):  conv0+leaky+W1 folds to
    y1 = faa*s + fbb*|s| with faa = alpha*f1, fbb = beta*f1 host-precomputed
    per-node broadcast tables (alpha = 0.505*sum(W0*W1),
    beta = 0.495*sum(|W0|*W1), f1 = (deg_out*deg_in)^-0.5): one ACT Abs plus
    three flat DVE ops.  (ACT Lrelu is NOT used -- its lowering is untested
    and produced wrong results on HW.)
  * h1 = leaky(deg_in^-0.5*agg1 + b1) via a dib broadcast table; the b1 add
    is skipped when b1 == 0 (plan-keyed).
  * MLP head matmuls run in bf16 (halves LDWEIGHTS time).
  * preps are spread one-per-SWDGE-queue (4 queues) so descriptor generation
    for all four 8192-token tiles runs in parallel (~64us wall instead of
    ~256us); per-queue ring capacity is dynamic_dma_scratch_size/4
    descriptors, so each queue holds at most one big prep.  The 1664-token
    tail tile runs as a normal gather after the triggers.

Round layout (unchanged from v1): round j holds the j-th in-edge of every
destination node, node-major (nodes degree-sorted descending within each
shard), padded to 128 tokens; the segment-sum is one full-width DVE add per
round-segment onto a resident SBUF accumulator.  Round 0 covers every node,
so it is a tensor_copy (no memset needed).
"""

import numpy as np
import ml_dtypes

N = 15828          # real node count (hardcoded per problem spec)
NP = 16384         # padded node count = 8 * 2048
S = 2048           # nodes per core shard
SJ = S // 128      # 16 shard blocks of 128 nodes
B = 64             # batch (propagation payload channels); 64 f32 = 256 bytes
NCORES = 8
HID = 100
TILE1 = 2048       # prop1 stream tile (tokens)
TILE2 = 8192       # prop2 gather tile (tokens, max per dma_gather call)
ZROW = S - 1       # guaranteed-pad (all-zero) table row in shard 0

NEG = 0.01
LA = (1.0 + NEG) / 2.0   # 0.505
LB = (1.0 - NEG) / 2.0   # 0.495


# ----------------------------------------------------------------------------
# Host-side graph preprocessing: pure index/layout work + per-node constants.
# ----------------------------------------------------------------------------

def _balance_nodes(deg_in):
    """Assign nodes to 8 bins of <=2048 balancing total in-edges, then order
    each bin by in-degree descending.  Returns new_label[old] in [0, NP)."""
    order = np.argsort(-deg_in, kind="stable")
    new_label = np.empty(N, dtype=np.int64)
    pos = np.zeros(NCORES, dtype=np.int64)
    edges = np.zeros(NCORES, dtype=np.int64)
    for start in range(0, N, NCORES):
        blk = order[start:start + NCORES]
        bins = np.argsort(edges, kind="stable")
        for i, n in enumerate(blk):
            k = int(bins[i])
            new_label[n] = k * S + pos[k]
            pos[k] += 1
            edges[k] += deg_in[n]
    assert pos.max() <= ZROW, pos  # keep ZROW free as the zero pad row
    return new_label


def _idx_layout(v, cols):
    """Pack int token-index vector v (len = 16*cols) into the SWDGE idx
    layout: [16, cols] with token i at [i % 16, i // 16], replicated to
    128 partitions."""
    a = np.asarray(v, dtype=np.int16).reshape(cols, 16).T  # [16, cols]
    return np.tile(a, (NCORES, 1)).copy()  # [128, cols]


def _bcast_nodes(vals):
    """Per-shard-node vector [S] -> [128, SJ*B] f32 broadcast table matching
    the agg tile layout (node = 128*j + p at [p, j, :])."""
    a = vals.reshape(SJ, 128).T.astype(np.float32)           # [128, SJ]
    return np.repeat(a[:, :, None], B, axis=2).reshape(128, SJ * B).copy()


def _prep(in_feat, edge_index, W0, b0, W1, b1, lw0, lb0, lw2, lb2, lw3, lb3):
    assert not np.asarray(b0).any(), "kernel assumes b0 == 0 (GCN collapse)"
    src = np.asarray(edge_index[0], dtype=np.int64)
    dst = np.asarray(edge_index[1], dtype=np.int64)

    deg_out = np.maximum(np.bincount(src, minlength=N), 1)
    deg_in = np.maximum(np.bincount(dst, minlength=N), 1)

    new_label = _balance_nodes(deg_in.copy())
    src_n = new_label[src]
    dst_n = new_label[dst]

    # padded per-node arrays in new labels
    xs0 = np.zeros((NP, B), dtype=np.float32)
    xs0[new_label] = (np.asarray(in_feat, dtype=np.float32)[:, :, 0]
                      * (deg_out.astype(np.float64) ** -0.5)[:, None])
    dego = np.ones(NP, dtype=np.float64)
    dego[new_label] = deg_out
    degi = np.ones(NP, dtype=np.float64)
    degi[new_label] = deg_in
    lw0n = np.zeros((HID, NP), dtype=np.float32)
    lw0n[:, new_label] = np.asarray(lw0, dtype=np.float32)

    # ---- round-layout token streams ----
    csr = []       # per core: (indptr[S+1], srcs sorted by dst)
    for k in range(NCORES):
        m = (dst_n // S) == k
        dk = dst_n[m] - k * S
        sk = src_n[m]
        o = np.argsort(dk, kind="stable")
        dk, sk = dk[o], sk[o]
        indptr = np.zeros(S + 1, dtype=np.int64)
        np.add.at(indptr, dk + 1, 1)
        indptr = np.cumsum(indptr)
        csr.append((indptr, sk))

    degs_local = [np.diff(c[0]) for c in csr]
    maxdeg = int(max(d.max() for d in degs_local))
    Mhat = [max(int((d > j).sum()) for d in degs_local) for j in range(maxdeg)]
    C = [-(-m // 128) for m in Mhat]      # round width in 128-token blocks
    rb = np.concatenate([[0], np.cumsum(np.array(C) * 128)])  # round bases
    e_pad = int(rb[-1])

    def mk_tiles(tile):
        tiles = []
        off = 0
        while off < e_pad:
            tiles.append(int(min(tile, e_pad - off)))
            off += tile
        segs = []  # per tile: list of (msg_col_a, msg_col_b, agg_col, round_j)
        tcol = 0
        for tlen in tiles:
            t_lo, t_hi = tcol, tcol + tlen // 128
            out = []
            for j in range(maxdeg):
                r_lo, r_hi = int(rb[j]) // 128, int(rb[j + 1]) // 128
                a, b_ = max(r_lo, t_lo), min(r_hi, t_hi)
                if a < b_:
                    out.append((int(a - t_lo), int(b_ - t_lo),
                                int(a - r_lo), int(j)))
            segs.append(tuple(out))
            tcol = t_hi
        return tuple(tiles), tuple(segs)

    tiles1, segs1 = mk_tiles(TILE1)
    tiles2, segs2 = mk_tiles(TILE2)
    b1_zero = not np.asarray(b1).any()

    # GCN collapse scalars (host; value-dependent things ship as inputs)
    w0v = np.asarray(W0, dtype=np.float64).reshape(-1)
    w1v = np.asarray(W1, dtype=np.float64).reshape(-1)
    alpha = LA * float((w0v * w1v).sum())
    beta = LB * float((np.abs(w0v) * w1v).sum())
    apb = alpha + beta
    assert abs(apb) > 1e-12, "degenerate W0/W1 (alpha+beta == 0)"
    slope = (alpha - beta) / apb

    lw2T = np.zeros((128, HID), dtype=np.float32)
    lw2T[:HID] = np.asarray(lw2, dtype=np.float32).T
    lw3T = np.zeros((128, 16), dtype=np.float32)
    lw3T[:HID, :10] = np.asarray(lw3, dtype=np.float32).T

    lbias = np.zeros((128, 4), dtype=np.float32)
    lbias[:HID, 0] = np.asarray(lb0, dtype=np.float32)
    lbias[:HID, 1] = np.asarray(lb2, dtype=np.float32)
    lbias[:10, 2] = np.asarray(lb3, dtype=np.float32)

    svec = np.zeros((128, 8), dtype=np.float32)
    svec[:, 0] = np.float32(slope)
    svec[:, 1] = np.float32(np.asarray(b1).reshape(-1)[0])

    in_maps = []
    for k in range(NCORES):
        indptr, sk = csr[k]
        d = degs_local[k]
        tok = np.full(e_pad, ZROW, dtype=np.int64)
        for j in range(maxdeg):
            has = np.nonzero(d > j)[0]       # prefix of labels (deg-sorted)
            tok[int(rb[j]):int(rb[j]) + has.size] = sk[indptr[has] + j]

        # prop1 message stream: xs0[tok] in gather output layout
        # (token i -> partition i%128, column i//128)
        ms1 = xs0[tok].reshape(e_pad // 128, 128, B)
        ms1 = np.ascontiguousarray(ms1.transpose(1, 0, 2)).reshape(128, -1)

        gidx = _idx_layout(tok, e_pad // 16)

        g = np.arange(S) + k * S
        f1 = (dego[g] * degi[g]) ** -0.5
        faa = _bcast_nodes(alpha * f1)
        fbb = _bcast_nodes(beta * f1)
        dib = _bcast_nodes(degi[g] ** -0.5)

        blk = lw0n[:, k * S:(k + 1) * S].T          # [2048, 100]
        blk = blk.reshape(SJ, 128, HID).transpose(1, 0, 2).reshape(128, SJ * HID)

        in_maps.append({
            "gidx": gidx,
            "ms1": ms1,
            "faa": faa,
            "fbb": fbb,
            "dib": dib,
            "svec": svec,
            "lbias": lbias,
            "lw0T": np.ascontiguousarray(blk).astype(ml_dtypes.bfloat16),
            "lw2T": lw2T.astype(ml_dtypes.bfloat16),
            "lw3T": lw3T.astype(ml_dtypes.bfloat16),
        })
    return in_maps, (e_pad, tiles1, segs1, tiles2, segs2, b1_zero)


# ----------------------------------------------------------------------------
# Bass program
# ----------------------------------------------------------------------------

def _build(plan):
    import concourse.bacc as bacc
    import concourse.mybir as mybir
    import concourse.tile as tile

    e_pad, tiles1, segs1, tiles2, segs2, b1_zero = plan
    f32 = mybir.dt.float32
    f16 = mybir.dt.float16
    bf16 = mybir.dt.bfloat16
    i16 = mybir.dt.int16
    AL = mybir.AluOpType
    ACT = mybir.ActivationFunctionType
    icols = e_pad // 16

    # descriptor ring sized to hold all prop2 gather descriptors (2 per token)
    scratch = 73728
    nc = bacc.Bacc("TRN2", target_bir_lowering=False, debug=False,
                   num_devices=NCORES, num_swdge_queues=4,
                   dynamic_dma_scratch_size=scratch)

    gidx_d = nc.dram_tensor("gidx", [128, icols], i16, kind="ExternalInput")
    ms1_d = nc.dram_tensor("ms1", [128, (e_pad // 128) * B], f32,
                           kind="ExternalInput")
    faa_d = nc.dram_tensor("faa", [128, SJ * B], f32, kind="ExternalInput")
    fbb_d = nc.dram_tensor("fbb", [128, SJ * B], f32, kind="ExternalInput")
    dib_d = nc.dram_tensor("dib", [128, SJ * B], f32, kind="ExternalInput")
    svec_d = nc.dram_tensor("svec", [128, 8], f32, kind="ExternalInput")
    lbias_d = nc.dram_tensor("lbias", [128, 4], f32, kind="ExternalInput")
    lw0T_d = nc.dram_tensor("lw0T", [128, SJ * HID], bf16, kind="ExternalInput")
    lw2T_d = nc.dram_tensor("lw2T", [128, HID], bf16, kind="ExternalInput")
    lw3T_d = nc.dram_tensor("lw3T", [128, 16], bf16, kind="ExternalInput")
    out_d = nc.dram_tensor("out", [10, B], f32, kind="ExternalOutput")

    y1in_d = nc.dram_tensor("y1in", [S, B], f32)
    y1full_d = nc.dram_tensor("y1full", [NP, B], f32, addr_space="Shared")
    hpin_d = nc.dram_tensor("hpin", [HID, B], f32)
    hpout_d = nc.dram_tensor("hpout", [HID, B], f32, addr_space="Shared")

    groups = [list(range(NCORES))]

    with tile.TileContext(nc, trace_sim=False) as tc:
        with (
            tc.tile_pool(name="const", bufs=1) as cpool,
            tc.tile_pool(name="ms1", bufs=2) as fpool,
            tc.tile_pool(name="psum", bufs=1, space="PSUM") as ppool,
        ):
            # gather indices first: prop2 descriptor prep depends only on them
            gix = cpool.tile([128, icols], i16)
            nc.sync.dma_start(gix[:], gidx_d.ap())

            # prop2 gather destinations (dedicated tiles, one per tile)
            msg2 = [cpool.tile([128, tl // 128, B], f32, name=f"msg2_{t}")
                    for t, tl in enumerate(tiles2)]

            # ---- small constant loads (sync engine, overlap prep) ----
            faa = cpool.tile([128, SJ * B], f32)
            nc.sync.dma_start(faa[:], faa_d.ap())
            fbb = cpool.tile([128, SJ * B], f32)
            nc.sync.dma_start(fbb[:], fbb_d.ap())
            dib = cpool.tile([128, SJ * B], f32)
            nc.sync.dma_start(dib[:], dib_d.ap())
            sv = cpool.tile([128, 8], f32)
            nc.sync.dma_start(sv[:], svec_d.ap())
            lb_sb = cpool.tile([128, 4], f32)
            nc.sync.dma_start(lb_sb[:], lbias_d.ap())
            lw0T_sb = cpool.tile([128, SJ * HID], bf16)
            nc.sync.dma_start(lw0T_sb[:], lw0T_d.ap())
            lw2T_sb = cpool.tile([128, HID], bf16)
            nc.sync.dma_start(lw2T_sb[:], lw2T_d.ap())
            lw3T_sb = cpool.tile([128, 16], bf16)
            nc.sync.dma_start(lw3T_sb[:], lw3T_d.ap())

            def seg_reduce(agg, mt, segs_t):
                for (a, b_, c, j) in segs_t:
                    dstv = agg[:, c:c + (b_ - a), :]
                    if j == 0:
                        nc.vector.tensor_copy(dstv, mt[:, a:b_, :])
                    else:
                        nc.vector.tensor_tensor(dstv, dstv, mt[:, a:b_, :],
                                                AL.add)

            # ---- prop1: host-built message stream, no gather ----
            agg0 = cpool.tile([128, SJ, B], f32)
            tok = 0
            for t, tl in enumerate(tiles1):
                blk = tl // 128
                ft = fpool.tile([128, TILE1 // 128, B], f32, tag="ms1")
                nc.sync.dma_start(
                    ft[:, :blk, :].rearrange("p c b -> p (c b)"),
                    ms1_d.ap()[:, (tok // 128) * B:((tok + tl) // 128) * B])
                seg_reduce(agg0, ft, segs1[t])
                tok += tl

            # ---- y1 = faa*agg0 + fbb*|agg0| ----
            y1 = cpool.tile([128, SJ, B], f32)
            a0f = agg0[:].rearrange("p j b -> p (j b)")
            y1f = y1[:].rearrange("p j b -> p (j b)")
            tmp0 = cpool.tile([128, SJ * B], f32)
            nc.scalar.activation(tmp0[:], a0f, ACT.Abs)
            nc.vector.tensor_tensor(tmp0[:], tmp0[:], fbb[:], AL.mult)
            nc.vector.tensor_tensor(y1f, a0f, faa[:], AL.mult)
            nc.vector.tensor_tensor(y1f, y1f, tmp0[:], AL.add)
            nc.sync.dma_start(y1in_d.ap().rearrange("(j p) m -> p j m", p=128),
                              y1[:])

            nc.gpsimd.collective_compute(
                "AllGather", AL.bypass, replica_groups=groups,
                ins=[y1in_d.ap().opt()], outs=[y1full_d.ap().opt()])

            # prop2 gathers: plain (self-triggered) dma_gather AFTER the
            # AllGather.  The gather instruction's own read of y1full carries
            # the collective-completion wait (v2-proven, bit-stable numerics);
            # prepare_only+trigger_dma raced here because the Tile scheduler
            # hoists the dependency-free triggers above the collective, and
            # signals_writable deadlocks the SWDGE ucode.  Spreading tiles
            # over the 4 SWDGE queues still parallelizes descriptor
            # generation across Q7 workers.
            for t in range(len(tiles2)):
                tl = tiles2[t]
                tok0 = sum(tiles2[:t])
                nc.gpsimd.dma_gather(
                    msg2[t][:, :tl // 128, :], y1full_d.ap(),
                    gix[:, tok0 // 16:(tok0 + tl) // 16],
                    tl, tl, B, queue_num=t % 4, single_packet=False)

            # ---- prop2 segment sums ----
            agg1 = cpool.tile([128, SJ, B], f32)
            for t in range(len(tiles2)):
                seg_reduce(agg1, msg2[t], segs2[t])

            # ---- h1 = leaky(dib * agg1 + b1), emitted in bf16 ----
            def leaky_inplace(x_ap, tmp_ap):
                # x = LA*x + LB*|x|
                nc.scalar.activation(tmp_ap, x_ap, ACT.Abs)
                nc.vector.tensor_scalar(tmp_ap, tmp_ap, float(LB), None,
                                        AL.mult)
                nc.vector.tensor_scalar(x_ap, x_ap, float(LA), None, AL.mult)
                nc.vector.tensor_tensor(x_ap, x_ap, tmp_ap, AL.add)

            tmp = cpool.tile([128, SJ * B], f32)
            a1f = agg1[:].rearrange("p j b -> p (j b)")
            nc.vector.tensor_tensor(tmp[:], a1f, dib[:], AL.mult)
            if not b1_zero:
                nc.vector.tensor_scalar(tmp[:], tmp[:], sv[:, 1:2], None,
                                        AL.add)
            leaky_inplace(tmp[:], tmp0[:])
            h1b = cpool.tile([128, SJ, B], bf16)
            h1bf = h1b[:].rearrange("p j b -> p (j b)")
            nc.vector.tensor_copy(h1bf, tmp[:])

            # ---- head: partial = sum_n lw0T[n,:]^T outer h1[n,:] ----
            ps = ppool.tile([HID, B], f32)
            for j in range(SJ):
                nc.tensor.matmul(ps[:], lhsT=lw0T_sb[:, j * HID:(j + 1) * HID],
                                 rhs=h1b[:, j, :], start=(j == 0),
                                 stop=(j == SJ - 1))
            hp = cpool.tile([HID, B], f32)
            nc.vector.tensor_copy(hp[:], ps[:])
            nc.sync.dma_start(hpin_d.ap(), hp[:])
            nc.gpsimd.collective_compute(
                "AllReduce", AL.add, replica_groups=groups,
                ins=[hpin_d.ap().opt()], outs=[hpout_d.ap().opt()])

            tz = cpool.tile([HID, B], f32)
            z0_32 = cpool.tile([HID, B], f32)
            nc.sync.dma_start(z0_32[:], hpout_d.ap())
            nc.vector.tensor_scalar(z0_32[:], z0_32[:], lb_sb[:HID, 0:1],
                                    None, AL.add)
            leaky_inplace(z0_32[:], tz[:])
            z0 = cpool.tile([HID, B], bf16)
            nc.vector.tensor_copy(z0[:], z0_32[:])

            ps2 = ppool.tile([HID, B], f32)
            nc.tensor.matmul(ps2[:], lhsT=lw2T_sb[:HID, :], rhs=z0[:],
                             start=True, stop=True)
            z1_32 = cpool.tile([HID, B], f32)
            nc.vector.tensor_copy(z1_32[:], ps2[:])
            nc.vector.tensor_scalar(z1_32[:], z1_32[:], lb_sb[:HID, 1:2],
                                    None, AL.add)
            leaky_inplace(z1_32[:], tz[:])
            z1 = cpool.tile([HID, B], bf16)
            nc.vector.tensor_copy(z1[:], z1_32[:])

            ps3 = ppool.tile([10, B], f32)
            nc.tensor.matmul(ps3[:], lhsT=lw3T_sb[:HID, 0:10], rhs=z1[:],
                             start=True, stop=True)
            z2 = cpool.tile([10, B], f32)
            nc.vector.tensor_copy(z2[:], ps3[:])
            nc.vector.tensor_scalar(z2[:], z2[:], lb_sb[:10, 2:3], None,
                                    AL.add)
            tz2 = cpool.tile([10, B], f32)
            leaky_inplace(z2[:], tz2[:])
            nc.sync.dma_start(out_d.ap(), z2[:])

    nc.compile()
    return nc


_BUILD_CACHE = {}
LAST_RESULTS = None  # BassKernelResults from the most recent run (for test.py)
RUN_KWARGS = {}      # extra kwargs for run_bass_kernel_spmd (test.py may set trace)


def kernel(**inputs) -> np.ndarray:
    global LAST_RESULTS
    from concourse.bass_utils import run_bass_kernel_spmd

    in_maps, plan = _prep(**inputs)
    if plan not in _BUILD_CACHE:
        _BUILD_CACHE[plan] = _build(plan)
    nc = _BUILD_CACHE[plan]

    res = run_bass_kernel_spmd(nc, in_maps, core_ids=list(range(NCORES)),
                               **RUN_KWARGS)
    LAST_RESULTS = res
    out = res.results[0]["out"]  # [10, 64]
    return np.ascontiguousarray(out.T.astype(np.float32))
